# revision 1
# baseline (speedup 1.0000x reference)
"""Trainium2 Bass kernel for the pairwise-MLP geometric convolution.

Reference computes, per batch z:
    rel[a,b]   = g[b] - g[a]
    h[a,b,:]   = relu(rel @ W1 + b1)                      [N,N,H]
    k[a,b,:]   = h @ W2 + b2  -> [N,N,C_OUT,C_IN]
    out[a,i]   = sum_{b,j} k[a,b,i,j] * f[b,j]

Key factorization (avoids materializing k):
    Ua  = g @ W1                (rel@W1 + b1 = Ub' - Ua, b1 folded into Ub')
    Ub' = [g_q, 1] @ [W1; b1]
    G[b,h,i] = sum_j W2[h, i*C_IN+j] * f[b,j]
    out[a,i] = sum_{b,h} relu(Ub'[b,h] - Ua[a,h]) * G[b,h,i]
             + sum_j b2[i,j] * (sum_b f[b,j])

Sharding over 8 cores: z (2) x b-quarter (4). Each core computes the full
[a=256, i=32] partial for its 64 b's; host sums quarters.

Layout trick that avoids any DRAM-bounce regroup of G: G is computed
TRANSPOSED, one matmul per output channel i, with the W2 slice as the
stationary operand:
    g_ps[h, i*64+b] = sum_j m2p2[j, i*64+h] * fTq[j, b]
so h lands on partitions directly. Strided partition-window copies then build
g3[(hl,h), p, i] = G[2p+hl, h, i]  (hl = b parity) in SBUF, and the main
contraction runs as 64 accumulating matmuls with K = (hl,h) = 128:
    acc[a_half, i] += t_p[:, a_half]^T @ g3[:, p, :]
where t_p[(hl,h), a] = relu(Ub'[2p+hl,h] - Ua[a,h]) is one tensor_scalar per
b-pair (bf16 in/out -> 4x DVE mode). The b2 bias is accumulated into its own
PSUM tile off the critical path and added during the output copy.

Hardware constraint honored throughout: a TPB instruction can carry at most
ONE sync-wait (walrus codegen limit). Shared T inputs are placed so every
consumer needs one wait (ub2 on DVE, negua2 halves on DVE+ACT, a Pool fence
op), and dummy PE matmuls observe the g3 copy semaphores before the main
accumulation chain.
"""

import os
import sys

import numpy as np

_TRN_REPO = "/opt/trn_rl_repo"
if _TRN_REPO not in sys.path:
    sys.path.insert(0, _TRN_REPO)

from contextlib import ExitStack

import concourse.bass as bass
import concourse.mybir as mybir
import concourse.tile as tile
from concourse.bass_utils import run_bass_kernel_spmd

from concourse.vector_clock import ScopedClock

# The walrus codegen used on the axon/PJRT path accepts at most ONE sync-wait
# per TPB instruction. Tile's kernel-tail drain aggregates a wait for every
# live semaphore onto a single Drain, which walrus rejects. Patch the tail to
# spread those waits across single-wait SP nops before an unadorned drain.
_orig_drain_and_barrier = tile.TileContext._drain_and_barrier


def _split_wait_drain_and_barrier(self, tick_clock, wait_clock):
    nc = self.nc
    probe = nc.sync.nop(nofuse=True)
    wait_clock.add_sem_waits(probe.ins, ScopedClock({None: tick_clock.global_clock}))
    si = probe.ins.sync_info
    waits = list(si.on_wait) if si is not None and si.on_wait else []
    if len(waits) > 1:
        probe.ins.sync_info = mybir.SyncInfo(on_wait=waits[:1], on_update=[])
        for w in waits[1:]:
            extra = nc.sync.nop(nofuse=True)
            extra.ins.sync_info = mybir.SyncInfo(on_wait=[w], on_update=[])
    nc.sync.drain()
    nc.all_engine_barrier()
    popped = nc._tile_sem_poison_stack.pop()
    assert popped is self._sem_poison
    nc.clear_and_free_semaphores(list(self.sems.allocated().values()))
    nc.all_engine_barrier()


tile.TileContext._drain_and_barrier = _split_wait_drain_and_barrier

F32 = mybir.dt.float32
BF16 = mybir.dt.bfloat16
Z, N, C_IN, C_OUT, H = 2, 256, 32, 32, 64
BQ = 64          # b-points per core (N / 4 quarters)
NPAIR = BQ // 2  # 32 K-chunks of (2 b x 64 h) = 128

# packed bf16 tensor (matmul operands) [32, MPW]:
#   cols 0:256      gT      (parts 0:3)   g[z].T for Ua
#   cols 256:320    gTb1    (parts 0:4)   [g[z,quarter].T; ones] for Ub'
#   cols 320:384    W1      (parts 0:3)
#   cols 384:448    W1b     (parts 0:4)   [W1; b1]
#   cols 448:576    ones    (part 0)      lhsT for the bias rank-1 matmuls
#   cols 576:640    fTq     (parts 0:32)  f[z,quarter].T
#   cols 640:2688   m2p2    (parts 0:32)  m2p2[j, i*64+h] = W2[h, i*C_IN+j]
#   cols 2688:2720  b2t     (parts 0:32)  b2[i*C_IN+j] transposed (q0 only)
MPW = 2720
D1A = 640  # first DMA: everything the U matmuls + bias pipeline needs

# engine for each of the 32 T-chunk builds: v=vector(DVE), g=gpsimd.
# ACT is saturated by the 8 g3 doubling copies.
T_ENGINES = [
    "v", "v", "v", "v", "g", "v", "v", "v",
    "v", "g", "v", "v", "v", "v", "g", "v",
    "v", "v", "v", "g", "v", "v", "v", "g",
    "v", "v", "v", "s", "v", "g", "v", "v",
]


def build_nc(debug: bool = False) -> bass.Bass:
    nc = bass.Bass("TRN2", target_bir_lowering=False, debug=debug, num_devices=8)

    mp = nc.dram_tensor("mp", [C_IN, MPW], BF16, kind="ExternalInput").ap()
    outp = nc.dram_tensor("outp", [N, C_OUT], F32, kind="ExternalOutput").ap()

    with tile.TileContext(nc) as tc, ExitStack() as ctx:
        consts = ctx.enter_context(tc.tile_pool(name="consts", bufs=1))
        work = ctx.enter_context(tc.tile_pool(name="work", bufs=1))
        # bufs=NPAIR: every T tile gets its own slot, so no T-op ever waits
        # for a PE slot release (keeps every instruction at <=1 sync wait).
        tpool = ctx.enter_context(tc.tile_pool(name="tpool", bufs=NPAIR))
        psum = ctx.enter_context(tc.tile_pool(name="psum", bufs=1, space="PSUM"))

        # ---- input loads: two SP HWDGE DMAs (U/bias inputs first, the big
        # m2p2 block second).
        mp_sb = consts.tile([C_IN, MPW], BF16)
        nc.sync.dma_start(out=mp_sb[:, 0:D1A], in_=mp[:, 0:D1A])
        nc.sync.dma_start(out=mp_sb[:, D1A:MPW], in_=mp[:, D1A:MPW])

        gT = mp_sb[0:3, 0:256]
        gTb1 = mp_sb[0:4, 256:320]
        w1 = mp_sb[0:3, 320:384]
        w1b = mp_sb[0:4, 384:448]
        ones_r = mp_sb[0:1, 448:576]
        fTq = mp_sb[:, 576:640]
        b2t_bf = mp_sb[:, 2688:2720]

        # ---- U matmuls: UaT = W1^T @ gT, UbT' = W1b^T @ [gq;1]  (bf16 in,
        # fp32 accumulate). One PSUM tile for both.
        u_ps = psum.tile([H, N + BQ], F32)
        uaT_ps = u_ps[:, 0:N]
        ubT_ps = u_ps[:, N:N + BQ]
        nc.tensor.matmul(uaT_ps, lhsT=w1, rhs=gT, start=True, stop=True)
        nc.tensor.matmul(ubT_ps, lhsT=w1b, rhs=gTb1, start=True, stop=True)

        # scol[j] = sum_{b in quarter} f[z,b,j] (host unshard completes the
        # b sum). First in the DVE queue: it only needs the first DMA and
        # runs inside the window where ub2 would wait on the U matmuls.
        scol = work.tile([C_IN, 1], BF16)
        with nc.allow_low_precision(reason="bias rank-1 term, tolerance 2e-2"):
            nc.vector.tensor_reduce(out=scol, in_=fTq,
                                    axis=mybir.AxisListType.X,
                                    op=mybir.AluOpType.add)

        # ALL shared T-op inputs live on DVE: the tile scheduler emits one
        # watermark wait per distinct producer engine, so a consumer on any
        # engine then needs exactly one (DVE) wait.
        # ub2[(hl,h), p] = Ub'[2p+hl, h]: strided column gather from ubT_ps.
        ub2 = work.tile([2 * H, NPAIR], F32)
        ubT_r = ubT_ps.rearrange("h (p two) -> h two p", two=2)
        nc.vector.tensor_copy(ub2[0:H, :], ubT_r[:, 0, :])
        nc.vector.tensor_copy(ub2[H:2 * H, :], ubT_r[:, 1, :])
        # negua2[(hl,h), a] = -Ua[a, h] on both partition halves (bf16 so the
        # DVE T-ops hit the 4x perf mode). The second half is a cheap
        # SBUF->SBUF bf16 copy of the first.
        negua2 = work.tile([2 * H, N], BF16)
        nc.vector.tensor_scalar(out=negua2[0:H, :], in0=uaT_ps,
                                scalar1=-1.0, scalar2=None,
                                op0=mybir.AluOpType.mult)
        nc.vector.tensor_copy(negua2[H:2 * H, :], negua2[0:H, :])
        # ---- G matmuls: g_ps[h, i*64+b] = sum_j m2p2[j, i*64+h] fTq[j, b].
        # One matmul per i; the W2 slice is the stationary operand so h lands
        # on partitions. Four 1-bank PSUM tiles, 8 i-slices each.
        gb = []
        for k in range(4):
            gp = psum.tile([BQ, 512], F32, name=f"g_ps{k}", tag=f"g_ps{k}")
            gb.append(gp)
        for i in range(C_OUT):
            k, off = divmod(i, 8)
            nc.tensor.matmul(gb[k][:, off * 64:(off + 1) * 64],
                             lhsT=mp_sb[:, 640 + i * 64:640 + (i + 1) * 64],
                             rhs=fTq, start=True, stop=True)

        # ---- T tiles: t_p[(hl,h), a] = relu(Ub'[2p+hl,h] - Ua[a,h]).
        t_tiles = []
        for p in range(NPAIR):
            t_p = tpool.tile([2 * H, N], BF16, tag="T", name=f"t_{p}")
            t_tiles.append(t_p)
            eng = T_ENGINES[p]
            if eng == "s":
                nc.scalar.activation(t_p, negua2,
                                     mybir.ActivationFunctionType.Relu,
                                     bias=ub2[:, p:p + 1], scale=1.0)
            else:
                e = nc.vector if eng == "v" else nc.gpsimd
                e.tensor_scalar(out=t_p, in0=negua2,
                                scalar1=ub2[:, p:p + 1], scalar2=0.0,
                                op0=mybir.AluOpType.add,
                                op1=mybir.AluOpType.max)

        # ---- partition-doubling copies on ACT: g3[(hl,h), p, i] =
        # G[2p+hl, h, i] = g_ps[h, i*64 + 2p+hl]. One strided copy per
        # (bank, hl) so each op carries a single PE-semaphore wait and starts
        # as soon as its bank's G matmuls are done.
        g3 = work.tile([2 * H, NPAIR, C_OUT], BF16)
        misc_ps = psum.tile([1, C_OUT + 1], F32)
        b2s_ps = misc_ps[:, 0:C_OUT]
        scrap = misc_ps[:, C_OUT:C_OUT + 1]
        b2s_sb = work.tile([1, C_OUT], BF16)

        def dbl(k, hl):
            srcv = gb[k].rearrange("h (i p two) -> h two p i", two=2, p=NPAIR)
            nc.scalar.activation(
                g3[hl * H:(hl + 1) * H, :, 8 * k:8 * (k + 1)],
                srcv[:, hl, :, :],
                mybir.ActivationFunctionType.Copy)

        dbl(0, 0)
        dbl(0, 1)
        # b2s[i] = sum_j b2t[j,i] scol[j] on PE right after the G matmuls
        # (which already observed the D1b DMA); the SBUF copy slots into the
        # ACT queue between g3 copies. By the time the bias enders run after
        # the mains, everything is long satisfied.
        nc.tensor.matmul(b2s_ps, lhsT=scol, rhs=b2t_bf, start=True, stop=True)
        nc.scalar.activation(b2s_sb, b2s_ps, mybir.ActivationFunctionType.Copy)
        for k in range(1, 4):
            for hl in range(2):
                dbl(k, hl)

        # PE observes the 8 g3 copy semaphores (one wait each) so the main
        # matmuls need only their T-tile wait.
        for k in range(4):
            for hl in range(2):
                nc.tensor.matmul(scrap,
                                 lhsT=g3[hl * H:(hl + 1) * H, 0, 8 * k:8 * k + 1],
                                 rhs=g3[hl * H:(hl + 1) * H, 0, 8 * k:8 * k + 1],
                                 start=True, stop=True)

        # ---- main contraction: acc[a_half, i] += t_p[:,half]^T @ g3[:,p,:]
        # Separate PSUM tiles per a-half: a start=True into a shared tile
        # wipes the other half's accumulation group. The bias matmul starts
        # each group so nothing bias-related sits on the tail.
        acc = [psum.tile([2 * H, C_OUT], F32, name=f"acc{i}", tag=f"acc{i}")
               for i in range(2)]
        for p in range(NPAIR):
            for half in range(2):
                nc.tensor.matmul(acc[half],
                                 lhsT=t_tiles[p][:, half * 2 * H:(half + 1) * 2 * H],
                                 rhs=g3[:, p, :],
                                 start=(p == 0), stop=False)

        # ---- bias enders: the final ops of each accumulation group; their
        # inputs were produced ~5us ago so they fire immediately:
        # acc[a, (half,i)] += ones^T @ b2s.
        for half in range(2):
            nc.tensor.matmul(acc[half], lhsT=ones_r, rhs=b2s_sb,
                             start=False, stop=True)

        # ---- store: out[a, i], a = half*128 + ap.
        out_sb = work.tile([2 * H, 2 * C_OUT], F32)
        nc.vector.tensor_copy(out_sb[:, 0:C_OUT], acc[0])
        nc.vector.tensor_copy(out_sb[:, C_OUT:2 * C_OUT], acc[1])
        srcv = bass.AP(tensor=out_sb.tensor, offset=out_sb.offset,
                       ap=[[2 * C_OUT, 2 * H], [C_OUT, 2], [1, C_OUT]])
        dstv = bass.AP(tensor=outp.tensor, offset=outp.offset,
                       ap=[[C_OUT, 2 * H], [2 * H * C_OUT, 2], [1, C_OUT]])
        nc.sync.dma_start(out=dstv, in_=srcv)

    return nc

def shard_inputs(features, geometry, W1, b1, W2, b2) -> list[dict]:
    import ml_dtypes
    bf16 = ml_dtypes.bfloat16
    f = np.ascontiguousarray(np.asarray(features, np.float32))
    g = np.ascontiguousarray(np.asarray(geometry, np.float32))
    W1 = np.ascontiguousarray(np.asarray(W1, np.float32))
    b1 = np.ascontiguousarray(np.asarray(b1, np.float32))
    W2 = np.ascontiguousarray(np.asarray(W2, np.float32))
    b2 = np.ascontiguousarray(np.asarray(b2, np.float32))

    # m2p2[j, i*64+h] = W2[h, i*C_IN+j]
    m2p2 = W2.reshape(H, C_OUT, C_IN).transpose(2, 1, 0).reshape(C_IN, C_OUT * H)
    b2t = np.ascontiguousarray(b2.reshape(C_OUT, C_IN).T)

    maps = []
    for core in range(8):
        z, q = divmod(core, 4)
        sl = slice(q * BQ, (q + 1) * BQ)
        mp = np.zeros((C_IN, MPW), bf16)
        mp[0:3, 0:256] = g[z].T.astype(bf16)
        mp[0:3, 256:320] = g[z, sl].T.astype(bf16)
        mp[3, 256:320] = 1.0
        mp[0:3, 320:384] = W1.astype(bf16)
        mp[0:3, 384:448] = W1.astype(bf16)
        mp[3, 384:448] = b1.astype(bf16)
        mp[0, 448:576] = 1.0
        mp[:, 576:640] = f[z, sl].T.astype(bf16)
        mp[:, 640:2688] = m2p2.astype(bf16)
        mp[:, 2688:2720] = b2t.astype(bf16)
        maps.append({"mp": mp})
    return maps


def unshard(parts: list[np.ndarray]) -> np.ndarray:
    out = np.empty((Z, N, C_OUT), np.float32)
    for z in range(Z):
        acc = parts[4 * z].astype(np.float32)
        for q in range(1, 4):
            acc = acc + parts[4 * z + q]
        out[z] = acc
    return out


def kernel(**inputs) -> np.ndarray:
    nc = build_nc(debug=False)
    in_maps = shard_inputs(**inputs)
    res = run_bass_kernel_spmd(nc, in_maps, list(range(8)))
    return unshard([r["outp"] for r in res.results])

